# revision 15
# baseline (speedup 1.0000x reference)
"""DGCNN (nn_DGCNN_param_57904749085240) Trainium2 Bass kernel.

Data-parallel over batch: 8 cores x 2 point clouds each, no collectives.

Per EdgeConv layer, instead of materializing (2C, N, k) edge features, use
    W @ [x_j - x_i; x_i] = W1 x_j + (W2 - W1) x_i
and eval-BN + leaky folding (per-channel scale s > 0 commutes with max_k):
    y[:, i] = leaky( max_{j in knn(i)} (A x_j)  +  Cc x_i + t )
with A = s*W1, Cc = s*(W2-W1), t = s*b + beta - s*mu, all host-folded.

knn: pd = 2 X^T X - xx_i - xx_j from the PE (fp16 inputs, fp32 PSUM; the
xx rows ride as 4 fp16 hi/lo-split aug rows so the xx term keeps ~fp32
precision).  Top-20 selection uses an index-encoded quantized key:
    q   = int16(pd * 2^s)            (Act copy, monotone)
    enc = float(q)*1024 + j          (Act upcast*1024, then Pool adds iota)
so 3x max8 + 2x match_replace on enc give 24 keys whose low 10 bits ARE
the column indices - no MaxIndex scans.  j = (int32)enc & 1023.
Neighbor max = gpsimd ap_gather (fp32) + grouped DVE tensor_reduce max,
pipelined inside the knn tile loop so DVE/Pool never drain at layer
boundaries.  All conv/lc/fc matmuls run in fp16 (4x faster PE) with the
lc/fc weights host-packed into partition-major resident SBUF tiles (no
jit weight DMAs on the critical path); leaky-relu is a single fused
scalar_tensor_tensor, with the lc mean-pool sum riding its accum_out.
"""
import sys

sys.path.insert(0, "/opt/trn_rl_repo")

import numpy as np

import concourse.bacc as bacc
import concourse.tile as tile
from concourse import mybir
from concourse.bass_utils import run_bass_kernel_spmd

F32 = mybir.dt.float32
F16 = mybir.dt.float16
I16 = mybir.dt.int16
I32 = mybir.dt.int32

B, N, K = 16, 1024, 20
N_CORES = 8
ELS = B // N_CORES
CH_C = [3, 64, 64, 128]
CH_O = [64, 64, 128, 256]
EMB = 1024
NT = N // 128
MMF = 512                     # matmul free-dim limit (one fp32 PSUM bank)
NEG = -1.0e30
# per-layer pd quantization scale 2^s: |pd|max*2^s*1024+1023 for the top-24
# candidates stays < 2^24 (exact fp32), int16 never saturates.
PDSCL = [256.0, 1024.0, 8192.0, 32768.0]
LC_ROWS = [(0, 64), (64, 128), (128, 256), (256, 384), (384, 512), (512, 513)]

AF = mybir.ActivationFunctionType
ALU = mybir.AluOpType
AX = mybir.AxisListType


def _mm(nc, out, lhsT, rhs, start, stop):
    fd = rhs.shape[-1]
    if fd <= MMF:
        nc.tensor.matmul(out=out, lhsT=lhsT, rhs=rhs, start=start, stop=stop)
        return
    for f0 in range(0, fd, MMF):
        f1 = min(f0 + MMF, fd)
        nc.tensor.matmul(out=out[:, f0:f1], lhsT=lhsT, rhs=rhs[:, f0:f1],
                         start=start, stop=stop)


def build_program(debug=False, reps=1, ablate=()):
    nc = bacc.Bacc("TRN2", target_bir_lowering=False, debug=False)

    x_in = nc.dram_tensor("x3", [ELS * 3, N], F16, kind="ExternalInput")
    iota_d = nc.dram_tensor("iota", [128, N], F32, kind="ExternalInput")
    wa_d, wc_d, wt_d = [], [], []
    for l in range(4):
        C, O = CH_C[l], CH_O[l]
        wa_d.append(nc.dram_tensor(f"wa{l}", [C, O], F16, kind="ExternalInput"))
        wc_d.append(nc.dram_tensor(f"wc{l}", [C, O], F16, kind="ExternalInput"))
        wt_d.append(nc.dram_tensor(f"wt{l}", [1, O], F16, kind="ExternalInput"))
    # host-packed partition-major weight blocks (see _fold_weights)
    wlc_d = nc.dram_tensor("wlc", [128, 8 * 6 * 128], F16, kind="ExternalInput")
    wl0_d = nc.dram_tensor("wl0", [128, 17 * 512], F16, kind="ExternalInput")
    wl1_d = nc.dram_tensor("wl1", [128, 5 * 256], F16, kind="ExternalInput")
    wow_d = nc.dram_tensor("wow", [128, 3 * 40], F16, kind="ExternalInput")
    out_d = nc.dram_tensor("out", [ELS, 40], F32, kind="ExternalOutput")

    with tile.TileContext(nc) as tc:
        with (
            tc.tile_pool(name="w", bufs=1) as wpool,
            tc.tile_pool(name="y", bufs=1) as ypool,
            tc.tile_pool(name="s1", bufs=1) as spool1,
            tc.tile_pool(name="s", bufs=2) as spool,
            tc.tile_pool(name="lc1", bufs=1) as lcpool,
            tc.tile_pool(name="pdp", bufs=3) as pdpool,
            tc.tile_pool(name="g", bufs=3) as gpool,
            tc.tile_pool(name="dr", bufs=2, space="DRAM") as dramp,
        ):
            # ---------------- consts + resident weights ----------------
            ones16 = wpool.tile([1, N], F16, tag="ones16")
            ones_col = wpool.tile([128, 1], F32, tag="ones_col")
            ones2 = wpool.tile([1, ELS], F16, tag="ones2")
            nc.vector.memset(ones16[:], 1.0)
            nc.vector.memset(ones_col[:], 1.0)
            nc.vector.memset(ones2[:], 1.0)
            iota_sb = wpool.tile([128, N], F32, tag="iota")
            nc.sync.dma_start(iota_sb[:], iota_d.ap())
            wlc_sb = wpool.tile([128, 8 * 6 * 128], F16, tag="wlc")
            nc.sync.dma_start(wlc_sb[:], wlc_d.ap())
            wl0_sb = wpool.tile([128, 17 * 512], F16, tag="wl0")
            nc.sync.dma_start(wl0_sb[:], wl0_d.ap())
            wl1_sb = wpool.tile([128, 5 * 256], F16, tag="wl1")
            nc.sync.dma_start(wl1_sb[:], wl1_d.ap())
            wow_sb = wpool.tile([128, 3 * 40], F16, tag="wow")
            nc.sync.dma_start(wow_sb[:], wow_d.ap())

            x0_tiles = []
            for el in range(ELS):
                t = ypool.tile([3, N], F16, tag=f"x0_{el}", name=f"x0_{el}")
                nc.sync.dma_start(t[:], x_in.ap()[el * 3:(el + 1) * 3, :])
                x0_tiles.append(t)

            wa, wc, wt = [], [], []
            for l in range(4):
                C, O = CH_C[l], CH_O[l]
                ta = wpool.tile([C, O], F16, tag=f"wa{l}")
                tcc = wpool.tile([C, O], F16, tag=f"wc{l}")
                tt = wpool.tile([1, O], F16, tag=f"wt{l}")
                nc.sync.dma_start(ta[:], wa_d[l].ap())
                nc.sync.dma_start(tcc[:], wc_d[l].ap())
                nc.sync.dma_start(tt[:], wt_d[l].ap())
                wa.append(ta); wc.append(tcc); wt.append(tt)

            # h_parts[l][el] = list of ([<=128, N] AP) feature chunks (lc order)
            h_parts = [[None] * ELS for _ in range(4)]
            maxes = ypool.tile([128, NT, ELS], F32, tag="maxes")
            sums = ypool.tile([128, NT, ELS], F32, tag="sums")
            # aug lhsT rows [-xxh; -xxl; 1; 1], raug rhs rows [1; 1; -xxh; -xxl]
            aug_t, raug_t = [], []
            for el in range(ELS):
                a1 = wpool.tile([4, N], F16, tag=f"aug{el}", name=f"aug{el}")
                a2 = wpool.tile([4, N], F16, tag=f"raug{el}", name=f"raug{el}")
                nc.sync.dma_start(a1[2:3, :], ones16[:])
                nc.sync.dma_start(a1[3:4, :], ones16[:])
                nc.sync.dma_start(a2[0:1, :], ones16[:])
                nc.sync.dma_start(a2[1:2, :], ones16[:])
                aug_t.append(a1); raug_t.append(a2)

            def prep_el(l, el, xf):
                """xx matmul + aug hi/lo rows + rhsf=2x for one cloud."""
                C = CH_C[l]
                xsq = spool1.tile([C, N], F32, tag=f"xsq{el}", name="xsq")
                nc.scalar.activation(out=xsq[:], in_=xf, func=AF.Square)
                xx_ps = psmm.tile([1, N], F32, tag="mm", name="xx_ps")
                _mm(nc, xx_ps[:], ones_col[0:C, :], xsq[:], True, True)
                aug, raug = aug_t[el], raug_t[el]
                nc.scalar.activation(out=aug[0:1, :], in_=xx_ps[:], func=AF.Copy,
                                     scale=-1.0)
                nhf = spool1.tile([1, N], F32, tag=f"nhf{el}", name="nhf")
                nc.scalar.activation(out=nhf[:], in_=aug[0:1, :], func=AF.Copy)
                nl = spool1.tile([1, N], F16, tag=f"nl{el}", name="nl")
                nc.vector.scalar_tensor_tensor(
                    out=nl[:], in0=nhf[:], scalar=-1.0, in1=xx_ps[:],
                    op0=ALU.mult, op1=ALU.subtract)
                nc.sync.dma_start(aug[1:2, :], nl[:])
                nc.sync.dma_start(raug[2:3, :], aug[0:1, :])
                nc.sync.dma_start(raug[3:4, :], nl[:])
                rhsf = spool1.tile([C, N], F16, tag=f"rhsf{el}", name="rhsf")
                nc.scalar.activation(out=rhsf[:], in_=xf, func=AF.Copy, scale=2.0)
                return rhsf

            def knn_tile(l, el, t, xf, rhsf, iw, p_base, nrep, flat):
                """pd matmul -> int16 quant -> +iota key -> top-24 -> idx dance."""
                aug, raug = aug_t[el], raug_t[el]
                pd_ps = pspd.tile([128, N], F32, tag="pd", name="pd_ps")
                _mm(nc, pd_ps[:], xf[:, t * 128:(t + 1) * 128], rhsf[:], True, False)
                _mm(nc, pd_ps[:], aug[:, t * 128:(t + 1) * 128], raug[:], False, True)
                pdi = pdpool.tile([128, N], I16, tag="pdi", name="pdi")
                nc.scalar.activation(out=pdi[:], in_=pd_ps[:], func=AF.Copy,
                                     scale=PDSCL[l])
                pdf = pdpool.tile([128, N], F32, tag="pdf", name="pdf")
                nc.scalar.activation(out=pdf[:], in_=pdi[:], func=AF.Copy,
                                     scale=1024.0)
                enc = pdpool.tile([128, N], F32, tag="enc", name="enc")
                nc.gpsimd.tensor_tensor(out=enc[:], in0=pdf[:], in1=iota_sb[:],
                                        op=ALU.add)
                v = pdpool.tile([128, 24], F32, tag="v", name="v")
                nc.vector.max(out=v[:, 0:8], in_=enc[:])
                nc.vector.match_replace(out=enc[:], in_to_replace=v[:, 0:8],
                                        in_values=enc[:], imm_value=NEG)
                nc.vector.max(out=v[:, 8:16], in_=enc[:])
                nc.vector.match_replace(out=enc[:], in_to_replace=v[:, 8:16],
                                        in_values=enc[:], imm_value=NEG)
                nc.vector.max(out=v[:, 16:24], in_=enc[:])
                vi = pdpool.tile([128, 24], I32, tag="vi", name="vi")
                nc.scalar.activation(out=vi[:], in_=v[:], func=AF.Copy)
                j32 = pdpool.tile([128, 24], I32, tag="j32", name="j32")
                nc.vector.tensor_scalar(out=j32[:], in0=vi[:], scalar1=1023,
                                        scalar2=None, op0=ALU.bitwise_and)
                j16 = pdpool.tile([128, 24], I16, tag="j16", name="j16")
                nc.vector.tensor_copy(j16[:], j32[:])
                c0, c1 = t * 160, (t + 1) * 160
                nc.sync.dma_start(flat[t * 128:(t + 1) * 128, :], j16[:, 0:K])
                src = (flat[t * 128:(t + 1) * 128, :]
                       .rearrange("p r -> (p r)")
                       .rearrange("(s w) -> w s", w=16))
                nc.sync.dma_start(iw[p_base:p_base + 16, c0:c1], src)
                blk = 16
                while blk < 16 * nrep:
                    nc.sync.dma_start(iw[p_base + blk:p_base + 2 * blk, c0:c1],
                                      iw[p_base:p_base + blk, c0:c1])
                    blk *= 2

            def gather_reduce(t, iw, a_sb, m_sb):
                g = gpool.tile([128, 2560], F32, tag="gath", name="g")
                nc.gpsimd.ap_gather(out_ap=g[:], in_ap=a_sb[:],
                                    idxs_ap=iw[:, t * 160:(t + 1) * 160],
                                    channels=128, num_elems=N, d=1, num_idxs=2560)
                nc.vector.tensor_reduce(
                    out=m_sb[:, t * 128:(t + 1) * 128],
                    in_=g[:].rearrange("p (i r) -> p i r", r=K),
                    axis=AX.X, op=ALU.max)

            for _rep in range(reps):
              Xf = [x0_tiles[el][:] for el in range(ELS)]
              with (
                tc.tile_pool(name=f"pspd{_rep}", bufs=3, space="PSUM") as pspd,
                tc.tile_pool(name=f"psmm{_rep}", bufs=1, space="PSUM") as psmm,
              ):
                # ================= EdgeConv layers =================
                for l in range(4):
                    C, O = CH_C[l], CH_O[l]
                    packed = (O == 64 and ELS == 2)
                    nch = 1 if packed else O // 128

                    if packed:
                        # conv mms first (independent of knn)
                        a_sb = spool.tile([128, N], F32, tag="asb", name="a_sb")
                        c_sb = spool.tile([128, N], F32, tag="csb", name="c_sb")
                        for el in range(ELS):
                            a_ps = psmm.tile([64, N], F32, tag="mm", name="a_ps")
                            _mm(nc, a_ps[:], wa[l][:, 0:O], Xf[el], True, True)
                            nc.scalar.activation(out=a_sb[64 * el:64 * (el + 1), :],
                                                 in_=a_ps[:], func=AF.Copy)
                            c_ps = psmm.tile([64, N], F32, tag="mm", name="c_ps")
                            _mm(nc, c_ps[:], wc[l][:, 0:O], Xf[el], True, False)
                            _mm(nc, c_ps[:], wt[l][:, 0:O], ones16[:], False, True)
                            nc.scalar.activation(out=c_sb[64 * el:64 * (el + 1), :],
                                                 in_=c_ps[:], func=AF.Copy)
                        rhsfs = [prep_el(l, el, Xf[el]) for el in range(ELS)]
                        iw = spool1.tile([128, NT * 160], I16, tag="iw0", name="iw")
                        flats = [dramp.tile([NT * 128, K], I16, tag=f"fl{el}",
                                            name=f"fl{el}") for el in range(ELS)]
                        m_sb = spool.tile([128, N], F32, tag="msb", name="m_sb")
                        for t in range(NT):
                            for el in range(ELS):
                                knn_tile(l, el, t, Xf[el], rhsfs[el], iw,
                                         64 * el, 4, flats[el])
                            gather_reduce(t, iw, a_sb, m_sb)
                        u = spool.tile([128, N], F32, tag="u", name="u")
                        nc.vector.tensor_tensor(out=u[:], in0=m_sb[:], in1=c_sb[:],
                                                op=ALU.add)
                        newX = []
                        for el in range(ELS):
                            yt = ypool.tile([64, N], F16, tag=f"y{l}_{el}",
                                            name=f"y{l}_{el}")
                            nc.vector.scalar_tensor_tensor(
                                out=yt[:], in0=u[64 * el:64 * (el + 1), :],
                                scalar=0.2, in1=u[64 * el:64 * (el + 1), :],
                                op0=ALU.mult, op1=ALU.max)
                            h_parts[l][el] = [yt[:]]
                            newX.append(yt[:])
                        Xf = newX
                    else:
                        newX = [None] * ELS
                        for el in range(ELS):
                            a_sbs, c_sbs, m_sbs = [], [], []
                            for ch in range(nch):
                                o0, o1 = ch * 128, (ch + 1) * 128
                                a_sb = spool.tile([128, N], F32, tag="asb",
                                                  name="a_sb")
                                a_ps = psmm.tile([128, N], F32, tag="mm",
                                                 name="a_ps")
                                _mm(nc, a_ps[:], wa[l][:, o0:o1], Xf[el], True, True)
                                nc.scalar.activation(out=a_sb[:], in_=a_ps[:],
                                                     func=AF.Copy)
                                c_ps = psmm.tile([128, N], F32, tag="mm",
                                                 name="c_ps")
                                _mm(nc, c_ps[:], wc[l][:, o0:o1], Xf[el], True, False)
                                _mm(nc, c_ps[:], wt[l][:, o0:o1], ones16[:],
                                    False, True)
                                c_sb = spool.tile([128, N], F32, tag="csb",
                                                  name="c_sb")
                                nc.scalar.activation(out=c_sb[:], in_=c_ps[:],
                                                     func=AF.Copy)
                                m_sb = spool.tile([128, N], F32, tag="msb",
                                                  name="m_sb")
                                a_sbs.append(a_sb); c_sbs.append(c_sb)
                                m_sbs.append(m_sb)
                            rhsf = prep_el(l, el, Xf[el])
                            iw = spool1.tile([128, NT * 160], I16, tag=f"iw{el}",
                                             name="iw")
                            flat = dramp.tile([NT * 128, K], I16, tag=f"fl{el}",
                                              name="flat")
                            for t in range(NT):
                                knn_tile(l, el, t, Xf[el], rhsf, iw, 0, 8, flat)
                                for ch in range(nch):
                                    gather_reduce(t, iw, a_sbs[ch], m_sbs[ch])
                            ychunks = []
                            for ch in range(nch):
                                u = spool.tile([128, N], F32, tag="u", name="u")
                                nc.vector.tensor_tensor(out=u[:], in0=m_sbs[ch][:],
                                                        in1=c_sbs[ch][:], op=ALU.add)
                                yt = ypool.tile([128, N], F16, tag=f"y{l}_{el}_{ch}",
                                                name="yt")
                                nc.vector.scalar_tensor_tensor(
                                    out=yt[:], in0=u[:], scalar=0.2, in1=u[:],
                                    op0=ALU.mult, op1=ALU.max)
                                ychunks.append(yt[:])
                            h_parts[l][el] = ychunks
                            if nch == 1:
                                newX[el] = ychunks[0]
                        if l < 3:
                            Xf = newX

                # ================= lc conv + pooling =================
                for el in range(ELS):
                    rhs_chunks = (h_parts[0][el] + h_parts[1][el] + h_parts[2][el]
                                  + h_parts[3][el] + [ones16[:]])
                    for mt in range(8):
                        u_ps = pspd.tile([128, N], F32, tag="pd", name="u_ps")
                        for kc in range(6):
                            r0, r1 = LC_ROWS[kc]
                            col = (mt * 6 + kc) * 128
                            _mm(nc, u_ps[:], wlc_sb[0:r1 - r0, col:col + 128],
                                rhs_chunks[kc], kc == 0, kc == 5)
                        u5 = lcpool.tile([128, N], F32, tag="u5", name="u5")
                        nc.scalar.activation(out=u5[:], in_=u_ps[:], func=AF.Copy)
                        y5 = lcpool.tile([128, N], F32, tag="y5", name="y5")
                        nc.vector.scalar_tensor_tensor(
                            out=y5[:], in0=u5[:], scalar=0.2, in1=u5[:],
                            op0=ALU.mult, op1=ALU.max,
                            accum_out=sums[:, mt:mt + 1, el:el + 1].rearrange(
                                "p a b -> p (a b)"))
                        nc.vector.tensor_reduce(out=maxes[:, mt:mt + 1, el:el + 1],
                                                in_=y5[:], axis=AX.X, op=ALU.max)

              # ================= FC head (els together as F=ELS) ==============
              mx16 = ypool.tile([128, NT, ELS], F16, tag="mx16")
              sm16 = ypool.tile([128, NT, ELS], F16, tag="sm16")
              nc.scalar.activation(out=mx16[:], in_=maxes[:], func=AF.Copy)
              nc.scalar.activation(out=sm16[:], in_=sums[:], func=AF.Copy)
              with tc.tile_pool(name=f"psfc{_rep}", bufs=1, space="PSUM") as psfc:
                  l0ps = [psfc.tile([128, ELS], F32, tag=f"fc{mt}", name=f"fc{mt}")
                          for mt in range(4)]
                  for kc in range(17):
                      nr = 128 if kc < 16 else 1
                      wj = wl0_sb[0:nr, kc * 512:(kc + 1) * 512]
                      if kc < 8:
                          rhs = mx16[:, kc:kc + 1, :].rearrange("p a b -> p (a b)")
                      elif kc < 16:
                          rhs = sm16[:, kc - 8:kc - 7, :].rearrange("p a b -> p (a b)")
                      else:
                          rhs = ones2[:]
                      for mt in range(4):
                          nc.tensor.matmul(out=l0ps[mt][:],
                                           lhsT=wj[:, mt * 128:(mt + 1) * 128],
                                           rhs=rhs, start=kc == 0, stop=kc == 16)
                  y6 = ypool.tile([128, 4 * ELS], F16, tag="y6")
                  y6v = y6[:].rearrange("p (a b) -> p a b", a=4)
                  for mt in range(4):
                      u6 = spool.tile([128, ELS], F32, tag="fcu", name="u6")
                      nc.scalar.activation(out=u6[:], in_=l0ps[mt][:], func=AF.Copy)
                      nc.vector.scalar_tensor_tensor(
                          out=y6v[:, mt:mt + 1, :].rearrange("p a b -> p (a b)"),
                          in0=u6[:], scalar=0.2, in1=u6[:],
                          op0=ALU.mult, op1=ALU.max)
                  l1ps = [psfc.tile([128, ELS], F32, tag=f"fd{mt}", name=f"fd{mt}")
                          for mt in range(2)]
                  for kc in range(5):
                      nr = 128 if kc < 4 else 1
                      wj = wl1_sb[0:nr, kc * 256:(kc + 1) * 256]
                      rhs = (y6v[:, kc:kc + 1, :].rearrange("p a b -> p (a b)")
                             if kc < 4 else ones2[:])
                      for mt in range(2):
                          nc.tensor.matmul(out=l1ps[mt][:],
                                           lhsT=wj[:, mt * 128:(mt + 1) * 128],
                                           rhs=rhs, start=kc == 0, stop=kc == 4)
                  y7 = ypool.tile([128, 2 * ELS], F16, tag="y7")
                  y7v = y7[:].rearrange("p (a b) -> p a b", a=2)
                  for mt in range(2):
                      u7 = spool.tile([128, ELS], F32, tag="fcu", name="u7")
                      nc.scalar.activation(out=u7[:], in_=l1ps[mt][:], func=AF.Copy)
                      nc.vector.scalar_tensor_tensor(
                          out=y7v[:, mt:mt + 1, :].rearrange("p a b -> p (a b)"),
                          in0=u7[:], scalar=0.2, in1=u7[:],
                          op0=ALU.mult, op1=ALU.max)
                  ops_ = psfc.tile([ELS, 40], F32, tag="fcout")
                  for kc in range(3):
                      if kc < 2:
                          lhsT = y7v[:, kc:kc + 1, :].rearrange("p a b -> p (a b)")
                          wj = wow_sb[0:128, kc * 40:(kc + 1) * 40]
                      else:
                          lhsT = ones2[:]
                          wj = wow_sb[0:1, 80:120]
                      nc.tensor.matmul(out=ops_[:], lhsT=lhsT, rhs=wj,
                                       start=kc == 0, stop=kc == 2)
                  osb = spool.tile([ELS, 40], F32, tag="osb")
                  nc.scalar.activation(out=osb[:], in_=ops_[:], func=AF.Copy)
                  nc.sync.dma_start(out_d.ap(), osb[:])

    nc.compile()
    return nc


def _pack_rows(w, chunk_rows, chunk_cols):
    """Pack [sum(chunk_rows), chunk_cols]-column blocks into [128, n*cols]."""
    out = np.zeros((128, len(chunk_rows) * chunk_cols), w.dtype)
    r0 = 0
    for i, nr in enumerate(chunk_rows):
        out[0:nr, i * chunk_cols:(i + 1) * chunk_cols] = w[r0:r0 + nr]
        r0 += nr
    return out


def _fold_weights(i):
    out = {}
    for l in range(4):
        C = CH_C[l]
        w = np.asarray(i[f"c{l}_w"], np.float64)
        b = np.asarray(i[f"c{l}_b"], np.float64)
        g = np.asarray(i[f"c{l}_g"], np.float64)
        be = np.asarray(i[f"c{l}_be"], np.float64)
        m = np.asarray(i[f"c{l}_m"], np.float64)
        v = np.asarray(i[f"c{l}_v"], np.float64)
        s = g / np.sqrt(v + 1e-5)
        w1, w2 = w[:, :C], w[:, C:]
        out[f"wa{l}"] = np.ascontiguousarray((s[:, None] * w1).T).astype(np.float16)
        out[f"wc{l}"] = np.ascontiguousarray((s[:, None] * (w2 - w1)).T).astype(np.float16)
        out[f"wt{l}"] = (s * b + be - s * m).astype(np.float16)[None, :]
    s = np.asarray(i["lc_g"], np.float64) / np.sqrt(np.asarray(i["lc_v"], np.float64) + 1e-5)
    t = s * np.asarray(i["lc_b"], np.float64) + np.asarray(i["lc_be"], np.float64) \
        - s * np.asarray(i["lc_m"], np.float64)
    wlc = np.concatenate([(s[:, None] * np.asarray(i["lc_w"], np.float64)).T,
                          t[None, :]], 0).astype(np.float16)   # [513, 1024]
    # wlc block (mt, kc): rows LC_ROWS[kc], cols mt*128:(mt+1)*128
    wlc_p = np.zeros((128, 8 * 6 * 128), np.float16)
    for mt in range(8):
        for kc, (r0, r1) in enumerate(LC_ROWS):
            col = (mt * 6 + kc) * 128
            wlc_p[0:r1 - r0, col:col + 128] = wlc[r0:r1, mt * 128:(mt + 1) * 128]
    out["wlc"] = wlc_p
    s = np.asarray(i["l0_g"], np.float64) / np.sqrt(np.asarray(i["l0_v"], np.float64) + 1e-5)
    t = np.asarray(i["l0_be"], np.float64) - s * np.asarray(i["l0_m"], np.float64)
    w = s[:, None] * np.asarray(i["l0_w"], np.float64)
    w[:, 1024:] /= 1024.0
    wl0 = np.concatenate([w.T, t[None, :]], 0).astype(np.float16)  # [2049, 512]
    out["wl0"] = _pack_rows(wl0, [128] * 16 + [1], 512)
    s = np.asarray(i["l1_g"], np.float64) / np.sqrt(np.asarray(i["l1_v"], np.float64) + 1e-5)
    t = s * np.asarray(i["l1_b"], np.float64) + np.asarray(i["l1_be"], np.float64) \
        - s * np.asarray(i["l1_m"], np.float64)
    wl1 = np.concatenate([(s[:, None] * np.asarray(i["l1_w"], np.float64)).T,
                          t[None, :]], 0).astype(np.float16)   # [513, 256]
    out["wl1"] = _pack_rows(wl1, [128] * 4 + [1], 256)
    wow = np.concatenate([np.asarray(i["ow"], np.float32).T,
                          np.asarray(i["ob"], np.float32)[None, :]],
                         0).astype(np.float16)                 # [257, 40]
    out["wow"] = _pack_rows(wow, [128, 128, 1], 40)
    out["iota"] = np.tile(np.arange(N, dtype=np.float32), (128, 1))
    return out


_NC_CACHE = {}


def get_program(debug=False):
    if debug not in _NC_CACHE:
        _NC_CACHE[debug] = build_program(debug)
    return _NC_CACHE[debug]


def make_in_maps(inputs):
    folded = _fold_weights(inputs)
    x = np.asarray(inputs["x"], np.float32)
    in_maps = []
    for c in range(N_CORES):
        m = dict(folded)
        xs = x[c * ELS:(c + 1) * ELS]                       # (ELS, 1024, 3)
        m["x3"] = np.ascontiguousarray(
            xs.transpose(0, 2, 1).reshape(ELS * 3, N)).astype(np.float16)
        in_maps.append(m)
    return in_maps


def kernel(**inputs) -> np.ndarray:
    nc = get_program(False)
    in_maps = make_in_maps(inputs)
    res = run_bass_kernel_spmd(nc, in_maps, list(range(N_CORES)))
    outs = [res.results[c]["out"] for c in range(N_CORES)]
    return np.concatenate(outs, 0).astype(np.float32)


# revision 17
# speedup vs baseline: 1.0428x; 1.0428x over previous
"""DGCNN (nn_DGCNN_param_57904749085240) Trainium2 Bass kernel.

Data-parallel over batch: 8 cores x 2 point clouds each, no collectives.

Per EdgeConv layer, instead of materializing (2C, N, k) edge features, use
    W @ [x_j - x_i; x_i] = W1 x_j + (W2 - W1) x_i
and eval-BN + leaky folding (per-channel scale s > 0 commutes with max_k):
    y[:, i] = leaky( max_{j in knn(i)} (A x_j)  +  Cc x_i + t )
with A = s*W1, Cc = s*(W2-W1), t = s*b + beta - s*mu, all host-folded.

knn: pd = 2 X^T X - xx_i - xx_j from the PE (fp16 inputs, fp32 PSUM; the
xx rows ride as 4 fp16 hi/lo-split aug rows so the xx term keeps ~fp32
precision).  Top-20 selection uses an index-encoded quantized key:
    q   = int16(pd * 2^s)            (Act copy, monotone)
    enc = float(q)*1024 + j          (Act upcast*1024, then Pool adds iota)
so 3x max8 + 2x match_replace on enc give 24 keys whose low 10 bits ARE
the column indices - no MaxIndex scans.  j = (int32)enc & 1023.
Neighbor max = gpsimd ap_gather (fp32) + grouped DVE tensor_reduce max,
pipelined inside the knn tile loop so DVE/Pool never drain at layer
boundaries.  All conv/lc/fc matmuls run in fp16 (4x faster PE) with the
lc/fc weights host-packed into partition-major resident SBUF tiles (no
jit weight DMAs on the critical path); leaky-relu is a single fused
scalar_tensor_tensor, with the lc mean-pool sum riding its accum_out.
"""
import sys

sys.path.insert(0, "/opt/trn_rl_repo")

import numpy as np

import concourse.bacc as bacc
import concourse.tile as tile
from concourse import mybir
from concourse.bass_utils import run_bass_kernel_spmd

F32 = mybir.dt.float32
F16 = mybir.dt.float16
I16 = mybir.dt.int16
I32 = mybir.dt.int32

B, N, K = 16, 1024, 20
N_CORES = 8
ELS = B // N_CORES
CH_C = [3, 64, 64, 128]
CH_O = [64, 64, 128, 256]
EMB = 1024
NT = N // 128
MMF = 512                     # matmul free-dim limit (one fp32 PSUM bank)
NEG = -1.0e30
# per-layer pd quantization scale 2^s: |pd|max*2^s*1024+1023 for the top-24
# candidates stays < 2^24 (exact fp32), int16 never saturates.
PDSCL = [256.0, 1024.0, 8192.0, 32768.0]
LC_ROWS = [(0, 64), (64, 128), (128, 256), (256, 384), (384, 512), (512, 513)]

AF = mybir.ActivationFunctionType
ALU = mybir.AluOpType
AX = mybir.AxisListType


def _mm(nc, out, lhsT, rhs, start, stop):
    fd = rhs.shape[-1]
    if fd <= MMF:
        nc.tensor.matmul(out=out, lhsT=lhsT, rhs=rhs, start=start, stop=stop)
        return
    for f0 in range(0, fd, MMF):
        f1 = min(f0 + MMF, fd)
        nc.tensor.matmul(out=out[:, f0:f1], lhsT=lhsT, rhs=rhs[:, f0:f1],
                         start=start, stop=stop)


def build_program(debug=False, reps=1, ablate=()):
    nc = bacc.Bacc("TRN2", target_bir_lowering=False, debug=False)

    x_in = nc.dram_tensor("x3", [ELS * 3, N], F16, kind="ExternalInput")
    iota_d = nc.dram_tensor("iota", [128, N], F32, kind="ExternalInput")
    wa_d, wc_d, wt_d = [], [], []
    for l in range(4):
        C, O = CH_C[l], CH_O[l]
        wa_d.append(nc.dram_tensor(f"wa{l}", [C, O], F16, kind="ExternalInput"))
        wc_d.append(nc.dram_tensor(f"wc{l}", [C, O], F16, kind="ExternalInput"))
        wt_d.append(nc.dram_tensor(f"wt{l}", [1, O], F16, kind="ExternalInput"))
    # host-packed partition-major weight blocks (see _fold_weights)
    wlc_d = nc.dram_tensor("wlc", [128, 8 * 6 * 128], F16, kind="ExternalInput")
    wl0_d = nc.dram_tensor("wl0", [128, 17 * 512], F16, kind="ExternalInput")
    wl1_d = nc.dram_tensor("wl1", [128, 5 * 256], F16, kind="ExternalInput")
    wow_d = nc.dram_tensor("wow", [128, 3 * 40], F16, kind="ExternalInput")
    out_d = nc.dram_tensor("out", [ELS, 40], F32, kind="ExternalOutput")

    with tile.TileContext(nc) as tc:
        with (
            tc.tile_pool(name="w", bufs=1) as wpool,
            tc.tile_pool(name="y", bufs=1) as ypool,
            tc.tile_pool(name="s1", bufs=1) as spool1,
            tc.tile_pool(name="s", bufs=2) as spool,
            tc.tile_pool(name="lc1", bufs=1) as lcpool,
            tc.tile_pool(name="pdp", bufs=3) as pdpool,
            tc.tile_pool(name="g", bufs=3) as gpool,
            tc.tile_pool(name="dr", bufs=2, space="DRAM") as dramp,
        ):
            # ---------------- consts + resident weights ----------------
            ones16 = wpool.tile([1, N], F16, tag="ones16")
            ones_col = wpool.tile([128, 1], F32, tag="ones_col")
            ones2 = wpool.tile([1, ELS], F16, tag="ones2")
            nc.vector.memset(ones16[:], 1.0)
            nc.vector.memset(ones_col[:], 1.0)
            nc.vector.memset(ones2[:], 1.0)
            iota_sb = wpool.tile([128, N], F32, tag="iota")
            nc.sync.dma_start(iota_sb[:], iota_d.ap())
            wlc_sb = wpool.tile([128, 8 * 6 * 128], F16, tag="wlc")
            nc.sync.dma_start(wlc_sb[:], wlc_d.ap())
            wl0_sb = wpool.tile([128, 17 * 512], F16, tag="wl0")
            nc.sync.dma_start(wl0_sb[:], wl0_d.ap())
            wl1_sb = wpool.tile([128, 5 * 256], F16, tag="wl1")
            nc.sync.dma_start(wl1_sb[:], wl1_d.ap())
            wow_sb = wpool.tile([128, 3 * 40], F16, tag="wow")
            nc.sync.dma_start(wow_sb[:], wow_d.ap())

            x0_tiles = []
            for el in range(ELS):
                t = ypool.tile([3, N], F16, tag=f"x0_{el}", name=f"x0_{el}")
                nc.sync.dma_start(t[:], x_in.ap()[el * 3:(el + 1) * 3, :])
                x0_tiles.append(t)

            wa, wc, wt = [], [], []
            for l in range(4):
                C, O = CH_C[l], CH_O[l]
                ta = wpool.tile([C, O], F16, tag=f"wa{l}")
                tcc = wpool.tile([C, O], F16, tag=f"wc{l}")
                tt = wpool.tile([1, O], F16, tag=f"wt{l}")
                nc.sync.dma_start(ta[:], wa_d[l].ap())
                nc.sync.dma_start(tcc[:], wc_d[l].ap())
                nc.sync.dma_start(tt[:], wt_d[l].ap())
                wa.append(ta); wc.append(tcc); wt.append(tt)

            # h_parts[l][el] = list of ([<=128, N] AP) feature chunks (lc order)
            h_parts = [[None] * ELS for _ in range(4)]
            maxes = ypool.tile([128, NT, ELS], F32, tag="maxes")
            sums = ypool.tile([128, NT, ELS], F32, tag="sums")
            # aug lhsT rows [-xxh; -xxl; 1; 1], raug rhs rows [1; 1; -xxh; -xxl]
            aug_t, raug_t = [], []
            for el in range(ELS):
                a1 = wpool.tile([4, N], F16, tag=f"aug{el}", name=f"aug{el}")
                a2 = wpool.tile([4, N], F16, tag=f"raug{el}", name=f"raug{el}")
                nc.sync.dma_start(a1[2:3, :], ones16[:])
                nc.sync.dma_start(a1[3:4, :], ones16[:])
                nc.sync.dma_start(a2[0:1, :], ones16[:])
                nc.sync.dma_start(a2[1:2, :], ones16[:])
                aug_t.append(a1); raug_t.append(a2)

            def prep_el(l, el, xf):
                """xx matmul + aug hi/lo rows + rhsf=2x for one cloud."""
                C = CH_C[l]
                xsq = spool1.tile([C, N], F32, tag=f"xsq{el}", name="xsq")
                nc.scalar.activation(out=xsq[:], in_=xf, func=AF.Square)
                xx_ps = psmm.tile([1, N], F32, tag="mm", name="xx_ps")
                _mm(nc, xx_ps[:], ones_col[0:C, :], xsq[:], True, True)
                aug, raug = aug_t[el], raug_t[el]
                nc.scalar.activation(out=aug[0:1, :], in_=xx_ps[:], func=AF.Copy,
                                     scale=-1.0)
                nhf = spool1.tile([1, N], F32, tag=f"nhf{el}", name="nhf")
                nc.scalar.activation(out=nhf[:], in_=aug[0:1, :], func=AF.Copy)
                nl = spool1.tile([1, N], F16, tag=f"nl{el}", name="nl")
                nc.vector.scalar_tensor_tensor(
                    out=nl[:], in0=nhf[:], scalar=-1.0, in1=xx_ps[:],
                    op0=ALU.mult, op1=ALU.subtract)
                nc.sync.dma_start(aug[1:2, :], nl[:])
                nc.sync.dma_start(raug[2:3, :], aug[0:1, :])
                nc.sync.dma_start(raug[3:4, :], nl[:])
                rhsf = spool1.tile([C, N], F16, tag=f"rhsf{el}", name="rhsf")
                nc.scalar.activation(out=rhsf[:], in_=xf, func=AF.Copy, scale=2.0)
                return rhsf

            def knn_tile(l, el, t, xf, rhsf, iw, p_base, nrep, flat):
                """pd matmul -> int16 quant -> +iota key -> top-24 -> idx dance."""
                aug, raug = aug_t[el], raug_t[el]
                pd_ps = pspd.tile([128, N], F32, tag="pd", name="pd_ps")
                _mm(nc, pd_ps[:], xf[:, t * 128:(t + 1) * 128], rhsf[:], True, False)
                _mm(nc, pd_ps[:], aug[:, t * 128:(t + 1) * 128], raug[:], False, True)
                pdi = pdpool.tile([128, N], I16, tag="pdi", name="pdi")
                nc.scalar.activation(out=pdi[:], in_=pd_ps[:], func=AF.Copy,
                                     scale=PDSCL[l])
                pdf = pdpool.tile([128, N], F32, tag="pdf", name="pdf")
                nc.scalar.activation(out=pdf[:], in_=pdi[:], func=AF.Copy,
                                     scale=1024.0)
                enc = pdpool.tile([128, N], F32, tag="enc", name="enc")
                nc.gpsimd.tensor_tensor(out=enc[:], in0=pdf[:], in1=iota_sb[:],
                                        op=ALU.add)
                v = pdpool.tile([128, 24], F32, tag="v", name="v")
                nc.vector.max(out=v[:, 0:8], in_=enc[:])
                nc.vector.match_replace(out=enc[:], in_to_replace=v[:, 0:8],
                                        in_values=enc[:], imm_value=NEG)
                nc.vector.max(out=v[:, 8:16], in_=enc[:])
                nc.vector.match_replace(out=enc[:], in_to_replace=v[:, 8:16],
                                        in_values=enc[:], imm_value=NEG)
                nc.vector.max(out=v[:, 16:24], in_=enc[:])
                vi = pdpool.tile([128, 24], I32, tag="vi", name="vi")
                nc.scalar.activation(out=vi[:], in_=v[:], func=AF.Copy)
                j32 = pdpool.tile([128, 24], I32, tag="j32", name="j32")
                nc.vector.tensor_scalar(out=j32[:], in0=vi[:], scalar1=1023,
                                        scalar2=None, op0=ALU.bitwise_and)
                j16 = pdpool.tile([128, 24], I16, tag="j16", name="j16")
                nc.vector.tensor_copy(j16[:], j32[:])
                c0, c1 = t * 160, (t + 1) * 160
                nc.sync.dma_start(flat[t * 128:(t + 1) * 128, :], j16[:, 0:K])
                src = (flat[t * 128:(t + 1) * 128, :]
                       .rearrange("p r -> (p r)")
                       .rearrange("(s w) -> w s", w=16))
                nc.sync.dma_start(iw[p_base:p_base + 16, c0:c1], src)
                blk = 16
                while blk < 16 * nrep:
                    nc.sync.dma_start(iw[p_base + blk:p_base + 2 * blk, c0:c1],
                                      iw[p_base:p_base + blk, c0:c1])
                    blk *= 2

            def gather_reduce(t, iw, a_sb, m_sb):
                g = gpool.tile([128, 2560], F32, tag="gath", name="g")
                nc.gpsimd.ap_gather(out_ap=g[:], in_ap=a_sb[:],
                                    idxs_ap=iw[:, t * 160:(t + 1) * 160],
                                    channels=128, num_elems=N, d=1, num_idxs=2560)
                nc.vector.tensor_reduce(
                    out=m_sb[:, t * 128:(t + 1) * 128],
                    in_=g[:].rearrange("p (i r) -> p i r", r=K),
                    axis=AX.X, op=ALU.max)

            for _rep in range(reps):
              Xf = [x0_tiles[el][:] for el in range(ELS)]
              with (
                tc.tile_pool(name=f"pspd{_rep}", bufs=3, space="PSUM") as pspd,
                tc.tile_pool(name=f"psmm{_rep}", bufs=1, space="PSUM") as psmm,
              ):
                # ================= EdgeConv layers =================
                for l in range(4):
                    C, O = CH_C[l], CH_O[l]
                    packed = (O == 64 and ELS == 2)
                    nch = 1 if packed else O // 128

                    if packed:
                        # conv mms first (independent of knn)
                        a_sb = spool.tile([128, N], F32, tag="asb", name="a_sb")
                        c_sb = spool.tile([128, N], F32, tag="csb", name="c_sb")
                        for el in range(ELS):
                            a_ps = psmm.tile([64, N], F32, tag="mm", name="a_ps")
                            _mm(nc, a_ps[:], wa[l][:, 0:O], Xf[el], True, True)
                            nc.scalar.activation(out=a_sb[64 * el:64 * (el + 1), :],
                                                 in_=a_ps[:], func=AF.Copy)
                            c_ps = psmm.tile([64, N], F32, tag="mm", name="c_ps")
                            _mm(nc, c_ps[:], wc[l][:, 0:O], Xf[el], True, False)
                            _mm(nc, c_ps[:], wt[l][:, 0:O], ones16[:], False, True)
                            nc.scalar.activation(out=c_sb[64 * el:64 * (el + 1), :],
                                                 in_=c_ps[:], func=AF.Copy)
                        rhsfs = [prep_el(l, el, Xf[el]) for el in range(ELS)]
                        iw = spool1.tile([128, NT * 160], I16, tag="iw0", name="iw")
                        flats = [dramp.tile([NT * 128, K], I16, tag=f"fl{el}",
                                            name=f"fl{el}") for el in range(ELS)]
                        m_sb = spool.tile([128, N], F32, tag="msb", name="m_sb")
                        LAG = 3
                        for t in range(NT + LAG):
                            if t < NT:
                                for el in range(ELS):
                                    knn_tile(l, el, t, Xf[el], rhsfs[el], iw,
                                             64 * el, 4, flats[el])
                            if t >= LAG:
                                gather_reduce(t - LAG, iw, a_sb, m_sb)
                        u = spool.tile([128, N], F32, tag="u", name="u")
                        nc.vector.tensor_tensor(out=u[:], in0=m_sb[:], in1=c_sb[:],
                                                op=ALU.add)
                        newX = []
                        for el in range(ELS):
                            yt = ypool.tile([64, N], F16, tag=f"y{l}_{el}",
                                            name=f"y{l}_{el}")
                            nc.vector.scalar_tensor_tensor(
                                out=yt[:], in0=u[64 * el:64 * (el + 1), :],
                                scalar=0.2, in1=u[64 * el:64 * (el + 1), :],
                                op0=ALU.mult, op1=ALU.max)
                            h_parts[l][el] = [yt[:]]
                            newX.append(yt[:])
                        Xf = newX
                    else:
                        newX = [None] * ELS
                        for el in range(ELS):
                            a_sbs, c_sbs, m_sbs = [], [], []
                            for ch in range(nch):
                                o0, o1 = ch * 128, (ch + 1) * 128
                                a_sb = spool.tile([128, N], F32, tag="asb",
                                                  name="a_sb")
                                a_ps = psmm.tile([128, N], F32, tag="mm",
                                                 name="a_ps")
                                _mm(nc, a_ps[:], wa[l][:, o0:o1], Xf[el], True, True)
                                nc.scalar.activation(out=a_sb[:], in_=a_ps[:],
                                                     func=AF.Copy)
                                c_ps = psmm.tile([128, N], F32, tag="mm",
                                                 name="c_ps")
                                _mm(nc, c_ps[:], wc[l][:, o0:o1], Xf[el], True, False)
                                _mm(nc, c_ps[:], wt[l][:, o0:o1], ones16[:],
                                    False, True)
                                c_sb = spool.tile([128, N], F32, tag="csb",
                                                  name="c_sb")
                                nc.scalar.activation(out=c_sb[:], in_=c_ps[:],
                                                     func=AF.Copy)
                                m_sb = spool.tile([128, N], F32, tag="msb",
                                                  name="m_sb")
                                a_sbs.append(a_sb); c_sbs.append(c_sb)
                                m_sbs.append(m_sb)
                            rhsf = prep_el(l, el, Xf[el])
                            iw = spool1.tile([128, NT * 160], I16, tag=f"iw{el}",
                                             name="iw")
                            flat = dramp.tile([NT * 128, K], I16, tag=f"fl{el}",
                                              name="flat")
                            LAG = 2 if nch > 1 else 3
                            for t in range(NT + LAG):
                                if t < NT:
                                    knn_tile(l, el, t, Xf[el], rhsf, iw, 0, 8, flat)
                                if t >= LAG:
                                    for ch in range(nch):
                                        gather_reduce(t - LAG, iw, a_sbs[ch],
                                                      m_sbs[ch])
                            ychunks = []
                            for ch in range(nch):
                                u = spool.tile([128, N], F32, tag="u", name="u")
                                nc.vector.tensor_tensor(out=u[:], in0=m_sbs[ch][:],
                                                        in1=c_sbs[ch][:], op=ALU.add)
                                yt = ypool.tile([128, N], F16, tag=f"y{l}_{el}_{ch}",
                                                name="yt")
                                nc.vector.scalar_tensor_tensor(
                                    out=yt[:], in0=u[:], scalar=0.2, in1=u[:],
                                    op0=ALU.mult, op1=ALU.max)
                                ychunks.append(yt[:])
                            h_parts[l][el] = ychunks
                            if nch == 1:
                                newX[el] = ychunks[0]
                        if l < 3:
                            Xf = newX

                # ================= lc conv + pooling =================
                for el in range(ELS):
                    rhs_chunks = (h_parts[0][el] + h_parts[1][el] + h_parts[2][el]
                                  + h_parts[3][el] + [ones16[:]])
                    for mt in range(8):
                        u_ps = pspd.tile([128, N], F32, tag="pd", name="u_ps")
                        for kc in range(6):
                            r0, r1 = LC_ROWS[kc]
                            col = (mt * 6 + kc) * 128
                            _mm(nc, u_ps[:], wlc_sb[0:r1 - r0, col:col + 128],
                                rhs_chunks[kc], kc == 0, kc == 5)
                        u5 = lcpool.tile([128, N], F32, tag="u5", name="u5")
                        nc.scalar.activation(out=u5[:], in_=u_ps[:], func=AF.Copy)
                        y5 = lcpool.tile([128, N], F32, tag="y5", name="y5")
                        nc.vector.scalar_tensor_tensor(
                            out=y5[:], in0=u5[:], scalar=0.2, in1=u5[:],
                            op0=ALU.mult, op1=ALU.max,
                            accum_out=sums[:, mt:mt + 1, el:el + 1].rearrange(
                                "p a b -> p (a b)"))
                        nc.vector.tensor_reduce(out=maxes[:, mt:mt + 1, el:el + 1],
                                                in_=y5[:], axis=AX.X, op=ALU.max)

              # ================= FC head (els together as F=ELS) ==============
              mx16 = ypool.tile([128, NT, ELS], F16, tag="mx16")
              sm16 = ypool.tile([128, NT, ELS], F16, tag="sm16")
              nc.scalar.activation(out=mx16[:], in_=maxes[:], func=AF.Copy)
              nc.scalar.activation(out=sm16[:], in_=sums[:], func=AF.Copy)
              with tc.tile_pool(name=f"psfc{_rep}", bufs=1, space="PSUM") as psfc:
                  l0ps = [psfc.tile([128, ELS], F32, tag=f"fc{mt}", name=f"fc{mt}")
                          for mt in range(4)]
                  for kc in range(17):
                      nr = 128 if kc < 16 else 1
                      wj = wl0_sb[0:nr, kc * 512:(kc + 1) * 512]
                      if kc < 8:
                          rhs = mx16[:, kc:kc + 1, :].rearrange("p a b -> p (a b)")
                      elif kc < 16:
                          rhs = sm16[:, kc - 8:kc - 7, :].rearrange("p a b -> p (a b)")
                      else:
                          rhs = ones2[:]
                      for mt in range(4):
                          nc.tensor.matmul(out=l0ps[mt][:],
                                           lhsT=wj[:, mt * 128:(mt + 1) * 128],
                                           rhs=rhs, start=kc == 0, stop=kc == 16)
                  y6 = ypool.tile([128, 4 * ELS], F16, tag="y6")
                  y6v = y6[:].rearrange("p (a b) -> p a b", a=4)
                  for mt in range(4):
                      u6 = spool.tile([128, ELS], F32, tag="fcu", name="u6")
                      nc.scalar.activation(out=u6[:], in_=l0ps[mt][:], func=AF.Copy)
                      nc.vector.scalar_tensor_tensor(
                          out=y6v[:, mt:mt + 1, :].rearrange("p a b -> p (a b)"),
                          in0=u6[:], scalar=0.2, in1=u6[:],
                          op0=ALU.mult, op1=ALU.max)
                  l1ps = [psfc.tile([128, ELS], F32, tag=f"fd{mt}", name=f"fd{mt}")
                          for mt in range(2)]
                  for kc in range(5):
                      nr = 128 if kc < 4 else 1
                      wj = wl1_sb[0:nr, kc * 256:(kc + 1) * 256]
                      rhs = (y6v[:, kc:kc + 1, :].rearrange("p a b -> p (a b)")
                             if kc < 4 else ones2[:])
                      for mt in range(2):
                          nc.tensor.matmul(out=l1ps[mt][:],
                                           lhsT=wj[:, mt * 128:(mt + 1) * 128],
                                           rhs=rhs, start=kc == 0, stop=kc == 4)
                  y7 = ypool.tile([128, 2 * ELS], F16, tag="y7")
                  y7v = y7[:].rearrange("p (a b) -> p a b", a=2)
                  for mt in range(2):
                      u7 = spool.tile([128, ELS], F32, tag="fcu", name="u7")
                      nc.scalar.activation(out=u7[:], in_=l1ps[mt][:], func=AF.Copy)
                      nc.vector.scalar_tensor_tensor(
                          out=y7v[:, mt:mt + 1, :].rearrange("p a b -> p (a b)"),
                          in0=u7[:], scalar=0.2, in1=u7[:],
                          op0=ALU.mult, op1=ALU.max)
                  ops_ = psfc.tile([ELS, 40], F32, tag="fcout")
                  for kc in range(3):
                      if kc < 2:
                          lhsT = y7v[:, kc:kc + 1, :].rearrange("p a b -> p (a b)")
                          wj = wow_sb[0:128, kc * 40:(kc + 1) * 40]
                      else:
                          lhsT = ones2[:]
                          wj = wow_sb[0:1, 80:120]
                      nc.tensor.matmul(out=ops_[:], lhsT=lhsT, rhs=wj,
                                       start=kc == 0, stop=kc == 2)
                  osb = spool.tile([ELS, 40], F32, tag="osb")
                  nc.scalar.activation(out=osb[:], in_=ops_[:], func=AF.Copy)
                  nc.sync.dma_start(out_d.ap(), osb[:])

    nc.compile()
    return nc


def _pack_rows(w, chunk_rows, chunk_cols):
    """Pack [sum(chunk_rows), chunk_cols]-column blocks into [128, n*cols]."""
    out = np.zeros((128, len(chunk_rows) * chunk_cols), w.dtype)
    r0 = 0
    for i, nr in enumerate(chunk_rows):
        out[0:nr, i * chunk_cols:(i + 1) * chunk_cols] = w[r0:r0 + nr]
        r0 += nr
    return out


def _fold_weights(i):
    out = {}
    for l in range(4):
        C = CH_C[l]
        w = np.asarray(i[f"c{l}_w"], np.float64)
        b = np.asarray(i[f"c{l}_b"], np.float64)
        g = np.asarray(i[f"c{l}_g"], np.float64)
        be = np.asarray(i[f"c{l}_be"], np.float64)
        m = np.asarray(i[f"c{l}_m"], np.float64)
        v = np.asarray(i[f"c{l}_v"], np.float64)
        s = g / np.sqrt(v + 1e-5)
        w1, w2 = w[:, :C], w[:, C:]
        out[f"wa{l}"] = np.ascontiguousarray((s[:, None] * w1).T).astype(np.float16)
        out[f"wc{l}"] = np.ascontiguousarray((s[:, None] * (w2 - w1)).T).astype(np.float16)
        out[f"wt{l}"] = (s * b + be - s * m).astype(np.float16)[None, :]
    s = np.asarray(i["lc_g"], np.float64) / np.sqrt(np.asarray(i["lc_v"], np.float64) + 1e-5)
    t = s * np.asarray(i["lc_b"], np.float64) + np.asarray(i["lc_be"], np.float64) \
        - s * np.asarray(i["lc_m"], np.float64)
    wlc = np.concatenate([(s[:, None] * np.asarray(i["lc_w"], np.float64)).T,
                          t[None, :]], 0).astype(np.float16)   # [513, 1024]
    # wlc block (mt, kc): rows LC_ROWS[kc], cols mt*128:(mt+1)*128
    wlc_p = np.zeros((128, 8 * 6 * 128), np.float16)
    for mt in range(8):
        for kc, (r0, r1) in enumerate(LC_ROWS):
            col = (mt * 6 + kc) * 128
            wlc_p[0:r1 - r0, col:col + 128] = wlc[r0:r1, mt * 128:(mt + 1) * 128]
    out["wlc"] = wlc_p
    s = np.asarray(i["l0_g"], np.float64) / np.sqrt(np.asarray(i["l0_v"], np.float64) + 1e-5)
    t = np.asarray(i["l0_be"], np.float64) - s * np.asarray(i["l0_m"], np.float64)
    w = s[:, None] * np.asarray(i["l0_w"], np.float64)
    w[:, 1024:] /= 1024.0
    wl0 = np.concatenate([w.T, t[None, :]], 0).astype(np.float16)  # [2049, 512]
    out["wl0"] = _pack_rows(wl0, [128] * 16 + [1], 512)
    s = np.asarray(i["l1_g"], np.float64) / np.sqrt(np.asarray(i["l1_v"], np.float64) + 1e-5)
    t = s * np.asarray(i["l1_b"], np.float64) + np.asarray(i["l1_be"], np.float64) \
        - s * np.asarray(i["l1_m"], np.float64)
    wl1 = np.concatenate([(s[:, None] * np.asarray(i["l1_w"], np.float64)).T,
                          t[None, :]], 0).astype(np.float16)   # [513, 256]
    out["wl1"] = _pack_rows(wl1, [128] * 4 + [1], 256)
    wow = np.concatenate([np.asarray(i["ow"], np.float32).T,
                          np.asarray(i["ob"], np.float32)[None, :]],
                         0).astype(np.float16)                 # [257, 40]
    out["wow"] = _pack_rows(wow, [128, 128, 1], 40)
    out["iota"] = np.tile(np.arange(N, dtype=np.float32), (128, 1))
    return out


_NC_CACHE = {}


def get_program(debug=False):
    if debug not in _NC_CACHE:
        _NC_CACHE[debug] = build_program(debug)
    return _NC_CACHE[debug]


def make_in_maps(inputs):
    folded = _fold_weights(inputs)
    x = np.asarray(inputs["x"], np.float32)
    in_maps = []
    for c in range(N_CORES):
        m = dict(folded)
        xs = x[c * ELS:(c + 1) * ELS]                       # (ELS, 1024, 3)
        m["x3"] = np.ascontiguousarray(
            xs.transpose(0, 2, 1).reshape(ELS * 3, N)).astype(np.float16)
        in_maps.append(m)
    return in_maps


def kernel(**inputs) -> np.ndarray:
    nc = get_program(False)
    in_maps = make_in_maps(inputs)
    res = run_bass_kernel_spmd(nc, in_maps, list(range(N_CORES)))
    outs = [res.results[c]["out"] for c in range(N_CORES)]
    return np.concatenate(outs, 0).astype(np.float32)
